# revision 24
# baseline (speedup 1.0000x reference)
"""Trainium2 Bass kernel for nn_MultiHeadAttention_85375359909998.

Causal MHA with (non-standard interleaved) RoPE, fp32 in/out.
  B=2, T=2048, D=1024, H=16, DH=64.

Sharding over 8 NeuronCores: data-parallel over batch (2) x tensor-parallel
over head groups (16 heads -> 4 groups of 4). Each core computes its batch's
QKV projection for its 4 heads, RoPE, causal attention, and a partial output
projection; the host sums the 4 partial projections per batch (the
"all-reduce") and concatenates batches.

Device-side layout notes (per core, heads grouped in pairs):
  - PE operands are fp16 (1 cycle/column streaming + fast weight loads);
    accumulation stays fp32 in PSUM.
  - q/k are produced *transposed* ([dh, t]) directly by the projection
    (host passes x^T and W^T); RoPE rotate-half is a 128x128 block-diagonal
    permutation matrix applied on the PE, combined with cos/sin muls split
    across DVE and GpSimd; evictions ride the Scalar engine
  - scores are computed transposed (S^T[s, t]) so the A@V matmul can use
    P^T tiles as the moving operand with V ([s, dh]) stationary; V gets an
    appended ones-column so row-sums (softmax denominators) fall out of the
    same matmul; the two heads of a pair run as concurrent 64x128 PE tiles
  - causal masking inside diagonal 128-blocks is a 0/1 triangular mask
    multiply on DVE applied to the exp'd tile (columns left of the block
    are never read by the A@V matmul, so they stay unmasked)
  - the A@V matmuls for group g are emitted after the scores of group g+1
    (software pipelining) so the PE never stalls waiting for the Scalar
    engine's exp eviction
  - softmax normalization: DVE fast-reciprocal straight from the PSUM sums
    row, GpSimd partition-broadcast, DVE multiply
  - partial projection outputs leave as fp16 (host accumulates in fp32)
"""

import sys
from contextlib import ExitStack

import numpy as np

try:
    import concourse.bass as bass  # noqa: F401
except ImportError:  # pragma: no cover
    sys.path.insert(0, "/opt/trn_rl_repo")
    import concourse.bass as bass  # noqa: F401

import concourse.tile as tile
from concourse import bacc, mybir
from concourse import bass_utils

B, T, D, H, DH = 2, 2048, 1024, 16, 64
NCORES = 8
GROUPS = 4          # head groups (tensor-parallel dimension)
HPC = H // GROUPS   # 4 heads per core
NPAIR = HPC // 2    # head pairs per core
TC512 = T // 512    # 4
SC128 = T // 128    # 16
KC = D // 128       # 8 contraction chunks for the projections
WARM = 12           # PE warm-up matmuls during the initial DMA wait

f32 = mybir.dt.float32
f16 = mybir.dt.float16
EXP = mybir.ActivationFunctionType.Exp

_CACHE = {}


def _rope_tables():
    """cos/sin tables, transposed & stacked for the [2*64, t] chunk layout."""
    inv = 1.0 / (10000.0 ** (np.arange(0, DH, 2, dtype=np.float64) / DH))  # 32
    t = np.arange(T, dtype=np.float64)
    freqs = t[:, None] * inv[None, :]                 # [T, 32]
    emb = np.concatenate([freqs, freqs], axis=-1)     # [T, 64]
    cos = np.cos(emb).astype(np.float32).T            # [64, T]
    sin = np.sin(emb).astype(np.float32).T
    csc = np.concatenate([cos, cos], axis=0)          # [128, T]
    csn = np.concatenate([sin, sin], axis=0)
    return (np.ascontiguousarray(csc.astype(np.float16)),
            np.ascontiguousarray(csn.astype(np.float16)))


def _rot_matrix():
    """R.T for rotate_half: (R@v)[2i] = -v[2i+1], (R@v)[2i+1] = v[2i]."""
    R = np.zeros((DH, DH), dtype=np.float32)
    for i in range(DH // 2):
        R[2 * i, 2 * i + 1] = -1.0
        R[2 * i + 1, 2 * i] = 1.0
    R128 = np.zeros((128, 128), dtype=np.float32)
    R128[:DH, :DH] = R
    R128[DH:, DH:] = R
    return np.ascontiguousarray(R128.T)


def _tri_mask():
    """tri[s, c] = 1 if c >= s else 0: keep-mask for diagonal 128-blocks."""
    sp = np.arange(128)[:, None]
    cp = np.arange(512)[None, :]
    return np.ascontiguousarray((cp >= sp).astype(np.float16))


def _emit(nc, tc, d, ctx):
    const = ctx.enter_context(tc.tile_pool(name="const", bufs=1))
    qkp = ctx.enter_context(tc.tile_pool(name="qk", bufs=1))
    vp = ctx.enter_context(tc.tile_pool(name="v", bufs=1))
    att = ctx.enter_context(tc.tile_pool(name="att", bufs=1))
    ptp = ctx.enter_context(tc.tile_pool(name="pt", bufs=4))
    tmp = ctx.enter_context(tc.tile_pool(name="tmp", bufs=3))
    small = ctx.enter_context(tc.tile_pool(name="small", bufs=2))
    stage = ctx.enter_context(tc.tile_pool(name="stage", bufs=4))
    # single PSUM pool, 8 banks total: "a" (qkv/v accum) 2, "s" (scores,
    # rope, warm-up) 4, "o0"/"o1" (attention-out accum, then proj) 2
    ps = ctx.enter_context(tc.tile_pool(name="ps", bufs=1, space="PSUM"))

    # ---- constants (DMA priority order: first-needed first, spread over
    #      the sync/scalar/gpsimd queues) ----
    warm_t = const.tile([128, 512], f16, tag="warm")
    nc.vector.memset(warm_t[:], 0.0)

    wqk_t = []
    for kc in range(KC):
        w1 = const.tile([128, 512], f16, tag=f"wqk{kc}", name=f"wqk{kc}")
        nc.sync.dma_start(w1[:], d["wqk"][128 * kc:128 * (kc + 1), :])
        wqk_t.append(w1)
    wv_t = []
    for kc in range(KC):
        w2 = const.tile([128, 256], f16, tag=f"wv{kc}", name=f"wv{kc}")
        nc.sync.dma_start(w2[:], d["wv"][128 * kc:128 * (kc + 1), :])
        wv_t.append(w2)

    rT_t = const.tile([128, 128], f16, tag="rT")
    csc_t = const.tile([128, T], f16, tag="csc")
    csn_t = const.tile([128, T], f16, tag="csn")
    tri_t = const.tile([128, 512], f16, tag="tri")
    wp_t = [const.tile([128, D], f16, tag=f"wp{kc}", name=f"wp{kc}")
            for kc in range(2)]
    nc.gpsimd.dma_start(rT_t[:], d["rT"][:])

    # x^T: first two t-chunks in [128, 512] pieces (they gate the first
    # matmuls), the second half in [128, 1024] pieces
    xT_t = [const.tile([128, T], f16, tag=f"xt{kc}", name=f"xt{kc}")
            for kc in range(KC)]
    for h2 in range(3):
        hsl2 = (slice(0, 512), slice(512, 1024), slice(1024, T))[h2]
        for kc in range(KC):
            dma_eng = nc.scalar if kc % 2 == 0 else nc.gpsimd
            dma_eng.dma_start(xT_t[kc][:, hsl2],
                              d["xT"][128 * kc:128 * (kc + 1), hsl2])
        nc.gpsimd.dma_start(csc_t[:, hsl2], d["csc"][:, hsl2])
        nc.gpsimd.dma_start(csn_t[:, hsl2], d["csn"][:, hsl2])
        if h2 == 0:
            nc.sync.dma_start(tri_t[:], d["tri"][:])
            for kc2 in range(2):
                nc.sync.dma_start(
                    wp_t[kc2][:], d["wp"][128 * kc2:128 * (kc2 + 1), :])

    # ---- persistent activations ----
    qk = [qkp.tile([128, T], f16, tag=f"qk{j}", name=f"qk{j}")
          for j in range(4)]
    # v layout per s-chunk: [V_h0 | ones | V_h1 | ones | ...] so the A@V
    # stationary [128, 128] = [V_h | ones]; the 64 ones columns make PSUM
    # rows 64-127 all equal to the softmax denominator (DVE-friendly).
    v_all = vp.tile([128, SC128 * 512], f16, tag="vall")
    nc.vector.memset(
        v_all.rearrange("p (n c) -> p n c", n=4 * SC128, c=128)[:, :, 64:128],
        1.0)
    attn = [att.tile([128, T], f16, tag=f"at{p}", name=f"at{p}")
            for p in range(NPAIR)]

    # warm-up matmuls: keep the PE array active (and the HAM clock
    # ungated) while the first x^T / weight DMAs stream in
    wps = ps.tile([128, 512], f32, tag="s", bufs=2, name="wps")
    for _ in range(WARM):
        nc.tensor.matmul(wps[:], warm_t[:, 0:128], warm_t[:],
                         start=True, stop=True)

    # ===== emission units (phase 1 and attention, interleaved below) ====
    def p1_qk(tcc, jp):
        """q/k projection block jp (2 of 4 qk rows) for t-chunk tcc + RoPE."""
        tsl = slice(512 * tcc, 512 * (tcc + 1))
        pss2 = [ps.tile([128, 512], f32, tag="a", bufs=2, name=f"ps{u}")
                for u in range(2)]
        for kc in range(KC):
            for u in range(2):
                jc = 2 * jp + u
                nc.tensor.matmul(pss2[u][:],
                                 wqk_t[kc][:, 128 * jc:128 * (jc + 1)],
                                 xT_t[kc][:, tsl],
                                 start=(kc == 0), stop=(kc == KC - 1))
        for u in range(2):
            jc = 2 * jp + u
            dst = qk[jc][:, tsl]
            nc.scalar.copy(dst, pss2[u][:])
            rps = ps.tile([128, 512], f32, tag="s", bufs=2, name="rps")
            nc.tensor.matmul(rps[:], rT_t[:], dst, start=True, stop=True)
            t1 = tmp.tile([128, 512], f32, tag="t1")
            nc.vector.tensor_mul(t1[:], rps[:], csn_t[:, tsl])
            t2 = tmp.tile([128, 512], f32, tag="t2")
            nc.gpsimd.tensor_mul(t2[:], dst, csc_t[:, tsl])
            nc.vector.tensor_add(dst, t1[:], t2[:])

    def p1_v(tcc):
        """V for the 4 s-chunks of t-chunk tcc, directly in [s, dh]."""
        for b4 in range(4):
            i = 4 * tcc + b4
            pv = ps.tile([128, 512], f32, tag="a", bufs=2, name="pv")
            for kc in range(KC):
                nc.tensor.matmul(pv[:, 0:256],
                                 xT_t[kc][:, 128 * i:128 * (i + 1)],
                                 wv_t[kc][:],
                                 start=(kc == 0), stop=(kc == KC - 1))
            nc.vector.tensor_copy(
                v_all[:, 512 * i:512 * (i + 1)].rearrange(
                    "p (a c) -> p a c", a=8, c=64)[:, 0::2, :],
                pv[:, 0:256].rearrange("p (a c) -> p a c", a=4, c=64))

    def pair(j, p):
        """Causal attention for head pair p over query strip j."""
        tsl = slice(512 * j, 512 * (j + 1))
        ni = 4 * (j + 1)
        qc = qk[2 * p]
        kch = qk[2 * p + 1]
        po = [ps.tile([128, 512], f32, tag=f"o{hh}", name=f"po{hh}")
              for hh in range(2)]

        def av(ptl, g):
            """A@V for one score group (emitted one group late)."""
            for hh in range(2):
                h = 2 * p + hh
                for half in range(2):
                    ii = 2 * g + half
                    diag = ii >= 4 * j
                    r = ii - 4 * j if diag else 0
                    c0 = 128 * r if diag else 0
                    nc.tensor.matmul(
                        po[hh][:, c0:512],
                        v_all[:, 512 * ii + 128 * h:512 * ii + 128 * (h + 1)],
                        ptl[hh][:, 512 * half + c0:512 * half + 512],
                        start=(ii == 0), stop=(ii == ni - 1))

        pending = []
        for g in range(ni // 2):
            pss = [ps.tile([128, 1024], f32, tag="s", bufs=2,
                           name=f"pss{hh}") for hh in range(2)]
            for half in range(2):
                ii = 2 * g + half
                diag = ii >= 4 * j
                r = ii - 4 * j if diag else 0
                c0 = 512 * half + (128 * r if diag else 0)
                qs = slice(512 * j + 128 * r, 512 * (j + 1)) \
                    if diag else tsl
                for hh in range(2):
                    hsl = slice(64 * hh, 64 * (hh + 1))
                    nc.tensor.matmul(
                        pss[hh][:, c0:512 * half + 512],
                        kch[hsl, 128 * ii:128 * (ii + 1)],
                        qc[hsl, qs],
                        start=True, stop=True)
            ptl = [ptp.tile([128, 1024], f16, tag=f"ptl{hh}",
                            name=f"ptl{hh}") for hh in range(2)]
            for hh in range(2):
                nc.scalar.activation(ptl[hh][:], pss[hh][:], EXP,
                                     scale=0.125)
            # zero the upper triangle of diagonal blocks (DVE)
            for half in range(2):
                ii = 2 * g + half
                if ii >= 4 * j:
                    r = ii - 4 * j
                    w = 512 - 128 * r
                    for hh in range(2):
                        reg = ptl[hh][:, 512 * half + 128 * r:
                                      512 * (half + 1)]
                        nc.vector.tensor_mul(reg, reg, tri_t[:, 0:w])
            pending.append((ptl, g))
            if len(pending) > 2:
                av(*pending.pop(0))
        for pe_ in pending:
            av(*pe_)

        for hh in range(2):
            sr = small.tile([64, 512], f32, tag="sr")
            nc.vector.tensor_copy(sr[:], po[hh][64:128, :])
            rc = small.tile([64, 512], f32, tag="rc")
            nc.vector.reciprocal_approx_fast(rc[:], sr[:])
            nc.vector.tensor_mul(
                attn[p][64 * hh:64 * (hh + 1), tsl],
                po[hh][0:64, :], rc[:])

    def proj(j):
        """Output projection (this core's partial) for strip j."""
        tsl = slice(512 * j, 512 * (j + 1))
        for oc in range(D // 128):
            pp = ps.tile([128, 512], f32, tag=f"o{oc % 2}", name="pp")
            for kc2 in range(2):
                nc.tensor.matmul(
                    pp[:], wp_t[kc2][:, 128 * oc:128 * (oc + 1)],
                    attn[kc2][:, tsl],
                    start=(kc2 == 0), stop=(kc2 == 1))
            ob = stage.tile([128, 512], f16, tag="ob")
            nc.vector.tensor_copy(ob[:], pp[:])
            out_eng = (nc.sync, nc.gpsimd, nc.scalar)[oc % 3]
            out_eng.dma_start(d["o"][128 * oc:128 * (oc + 1), tsl], ob[:])

    # ===== interleaved schedule: attention strip j runs between the =====
    # ===== projection blocks of t-chunk j+1 (PE work fills the gaps =====
    # ===== left by the Scalar engine's exp evictions, and vice versa) ===
    p1_qk(0, 0)
    p1_v(0)
    for j in range(TC512):
        pair(j, 0)
        p1_qk(j, 1) if j == 0 else None
        if j + 1 < TC512:
            p1_qk(j + 1, 0)
        pair(j, 1)
        if j + 1 < TC512:
            p1_qk(j + 1, 1)
        proj(j)
        if j + 1 < TC512:
            p1_v(j + 1)


def _build_module():
    nc = bacc.Bacc("TRN2", target_bir_lowering=False, debug=False,
                   enable_asserts=False)
    d = {
        "xT": nc.dram_tensor("xT", [D, T], f16, kind="ExternalInput").ap(),
        "wqk": nc.dram_tensor("wqk", [D, 512], f16, kind="ExternalInput").ap(),
        "wv": nc.dram_tensor("wv", [D, 256], f16, kind="ExternalInput").ap(),
        "wp": nc.dram_tensor("wp", [256, D], f16, kind="ExternalInput").ap(),
        "csc": nc.dram_tensor("csc", [128, T], f16, kind="ExternalInput").ap(),
        "csn": nc.dram_tensor("csn", [128, T], f16, kind="ExternalInput").ap(),
        "rT": nc.dram_tensor("rT", [128, 128], f16, kind="ExternalInput").ap(),
        "tri": nc.dram_tensor("tri", [128, 512], f16,
                              kind="ExternalInput").ap(),
        "o": nc.dram_tensor("o", [D, T], f16, kind="ExternalOutput").ap(),
    }
    with tile.TileContext(nc) as tc:
        with ExitStack() as ctx, \
             nc.allow_low_precision("fp16 PE operands are rounded by design"):
            _emit(nc, tc, d, ctx)
    nc.compile()
    return nc


def _get_module():
    if "nc" not in _CACHE:
        _CACHE["nc"] = _build_module()
    return _CACHE["nc"]


def _canonical(attn_mask, key_padding_mask):
    if attn_mask.shape != (1, 1, T, T) or key_padding_mask.shape != (B, T):
        return False
    if not key_padding_mask.all():
        return False
    m = np.asarray(attn_mask[0, 0], dtype=np.float32)
    causal = np.triu(np.full((T, T), -1e9, dtype=np.float32), k=1)
    return np.array_equal(m, causal)


def _reference_fallback(x, attn_mask, key_padding_mask, Wqkv, Wproj):
    x = np.asarray(x, np.float32)
    qkv = x @ np.asarray(Wqkv, np.float32).T
    q, k, v = qkv[..., :D], qkv[..., D:2 * D], qkv[..., 2 * D:]

    def split(t):
        return t.reshape(B, -1, H, DH).transpose(0, 2, 1, 3)

    def rope(xx):
        inv = 1.0 / (10000.0 ** (np.arange(0, DH, 2, dtype=np.float32) / DH))
        fr = np.arange(T, dtype=np.float32)[:, None] * inv[None, :]
        emb = np.concatenate([fr, fr], axis=-1)
        cos, sin = np.cos(emb)[None, None], np.sin(emb)[None, None]
        x1, x2 = xx[..., ::2], xx[..., 1::2]
        rh = np.stack((-x2, x1), axis=-1).reshape(xx.shape)
        return xx * cos + rh * sin

    q, k, v = split(q), split(k), split(v)
    q, k = rope(q), rope(k)
    s = np.einsum("bhtd,bhsd->bhts", q, k) / np.sqrt(np.float32(DH))
    s = s + np.asarray(attn_mask, np.float32)
    s = np.where(np.asarray(key_padding_mask)[:, None, None, :], s, -1e9)
    s = s - s.max(axis=-1, keepdims=True)
    e = np.exp(s)
    a = e / e.sum(axis=-1, keepdims=True)
    out = np.einsum("bhts,bhsd->bhtd", a, v)
    out = out.transpose(0, 2, 1, 3).reshape(B, T, D)
    return out @ np.asarray(Wproj, np.float32).T


def _make_in_maps(x, Wqkv, Wproj):
    csc, csn = _rope_tables()
    rT = _rot_matrix().astype(np.float16)
    tri = _tri_mask()

    Wq = np.asarray(Wqkv[:D], np.float32).reshape(H, DH, D)
    Wk = np.asarray(Wqkv[D:2 * D], np.float32).reshape(H, DH, D)
    Wv = np.asarray(Wqkv[2 * D:], np.float32).reshape(H, DH, D)
    WpT = np.ascontiguousarray(np.asarray(Wproj, np.float32).T)  # [din, dout]

    xT = [np.ascontiguousarray(np.asarray(x[b], np.float32).T.astype(np.float16))
          for b in range(B)]

    in_maps = []
    for c in range(NCORES):
        b, g = divmod(c, GROUPS)
        hs = [HPC * g + hl for hl in range(HPC)]  # global head ids
        cols = []
        for pp in range(NPAIR):
            h0, h1 = hs[2 * pp], hs[2 * pp + 1]
            cols.append(np.concatenate([Wq[h0], Wq[h1]], axis=0))  # [128, D]
            cols.append(np.concatenate([Wk[h0], Wk[h1]], axis=0))
        wqk = np.ascontiguousarray(
            np.concatenate(cols, axis=0).T.astype(np.float16))     # [D, 512]
        wv = np.ascontiguousarray(
            np.concatenate([Wv[h] for h in hs], axis=0).T.astype(np.float16))
        wp = np.ascontiguousarray(
            WpT[256 * g:256 * (g + 1), :].astype(np.float16))  # [256, D]
        in_maps.append({
            "xT": xT[b], "wqk": wqk, "wv": wv, "wp": wp,
            "csc": csc, "csn": csn, "rT": rT, "tri": tri,
        })
    return in_maps


def _in_maps_for_trace(inputs):
    return _make_in_maps(np.asarray(inputs["x"]), np.asarray(inputs["Wqkv"]),
                         np.asarray(inputs["Wproj"]))


def kernel(x, attn_mask, key_padding_mask, Wqkv, Wproj):
    x = np.asarray(x)
    attn_mask = np.asarray(attn_mask)
    key_padding_mask = np.asarray(key_padding_mask)
    Wqkv = np.asarray(Wqkv)
    Wproj = np.asarray(Wproj)

    if not _canonical(attn_mask, key_padding_mask):
        return _reference_fallback(x, attn_mask, key_padding_mask, Wqkv, Wproj)

    nc = _get_module()
    in_maps = _make_in_maps(x, Wqkv, Wproj)
    res = bass_utils.run_bass_kernel_spmd(nc, in_maps,
                                          core_ids=list(range(NCORES)))
    out = np.empty((B, T, D), dtype=np.float32)
    for b in range(B):
        acc = res.results[4 * b]["o"].astype(np.float32)
        for g in range(1, GROUPS):
            acc += res.results[4 * b + g]["o"].astype(np.float32)
        out[b] = acc.T
    return out


# revision 25
# speedup vs baseline: 1.0465x; 1.0465x over previous
"""Trainium2 Bass kernel for nn_MultiHeadAttention_85375359909998.

Causal MHA with (non-standard interleaved) RoPE, fp32 in/out.
  B=2, T=2048, D=1024, H=16, DH=64.

Sharding over 8 NeuronCores: data-parallel over batch (2) x tensor-parallel
over head groups (16 heads -> 4 groups of 4). Each core computes its batch's
QKV projection for its 4 heads, RoPE, causal attention, and a partial output
projection; the host sums the 4 partial projections per batch (the
"all-reduce") and concatenates batches.

Device-side layout notes (per core, heads grouped in pairs):
  - PE operands are fp16 (1 cycle/column streaming + fast weight loads);
    accumulation stays fp32 in PSUM.
  - q/k are produced *transposed* ([dh, t]) directly by the projection
    (host passes x^T and W^T); RoPE rotate-half is a 128x128 block-diagonal
    permutation matrix applied on the PE, combined with cos/sin muls split
    across DVE and GpSimd; evictions ride the Scalar engine
  - scores are computed transposed (S^T[s, t]) so the A@V matmul can use
    P^T tiles as the moving operand with V ([s, dh]) stationary; V carries
    64 appended ones-columns per head so PSUM rows 64-127 of the A@V
    output all equal the softmax denominator (normalization then needs no
    partition broadcast); the two heads of a pair run as concurrent
    64x128 PE tiles (auto row-tiling via base partitions)
  - causal masking inside diagonal 128-blocks is a 0/1 triangular mask
    multiply on DVE applied to the exp'd tile (columns left of the block
    are never read by the A@V matmul, so they stay unmasked)
  - the attention strips are interleaved with the projection blocks of
    the NEXT t-chunk, so the Scalar engine's exp stream overlaps the
    PE's projection matmuls; A@V for group g is emitted two score groups
    late (software pipelining)
  - PSUM (8 banks) is shared via pool tags: "a" qkv/v accumulators (2),
    "s" scores + RoPE + warm-up (4), "o0"/"o1" attention-out accumulators
    that the output projection reuses (2)
  - a warm-up matmul stream covers the initial DMA window so the PE HAM
    clock-gate is released before the first real matmuls
  - softmax normalization: DVE copy of the denominator rows, DVE
    fast-reciprocal, DVE multiply
  - partial projection outputs leave as fp16 (host accumulates in fp32)
"""

import sys
from contextlib import ExitStack

import numpy as np

try:
    import concourse.bass as bass  # noqa: F401
except ImportError:  # pragma: no cover
    sys.path.insert(0, "/opt/trn_rl_repo")
    import concourse.bass as bass  # noqa: F401

import concourse.tile as tile
from concourse import bacc, mybir
from concourse import bass_utils

B, T, D, H, DH = 2, 2048, 1024, 16, 64
NCORES = 8
GROUPS = 4          # head groups (tensor-parallel dimension)
HPC = H // GROUPS   # 4 heads per core
NPAIR = HPC // 2    # head pairs per core
TC512 = T // 512    # 4
SC128 = T // 128    # 16
KC = D // 128       # 8 contraction chunks for the projections
WARM = 12           # PE warm-up matmuls during the initial DMA wait

f32 = mybir.dt.float32
f16 = mybir.dt.float16
EXP = mybir.ActivationFunctionType.Exp

_CACHE = {}


def _rope_tables():
    """cos/sin tables, transposed & stacked for the [2*64, t] chunk layout."""
    inv = 1.0 / (10000.0 ** (np.arange(0, DH, 2, dtype=np.float64) / DH))  # 32
    t = np.arange(T, dtype=np.float64)
    freqs = t[:, None] * inv[None, :]                 # [T, 32]
    emb = np.concatenate([freqs, freqs], axis=-1)     # [T, 64]
    cos = np.cos(emb).astype(np.float32).T            # [64, T]
    sin = np.sin(emb).astype(np.float32).T
    csc = np.concatenate([cos, cos], axis=0)          # [128, T]
    csn = np.concatenate([sin, sin], axis=0)
    return (np.ascontiguousarray(csc.astype(np.float16)),
            np.ascontiguousarray(csn.astype(np.float16)))


def _rot_matrix():
    """R.T for rotate_half: (R@v)[2i] = -v[2i+1], (R@v)[2i+1] = v[2i]."""
    R = np.zeros((DH, DH), dtype=np.float32)
    for i in range(DH // 2):
        R[2 * i, 2 * i + 1] = -1.0
        R[2 * i + 1, 2 * i] = 1.0
    R128 = np.zeros((128, 128), dtype=np.float32)
    R128[:DH, :DH] = R
    R128[DH:, DH:] = R
    return np.ascontiguousarray(R128.T)


def _tri_mask():
    """tri[s, c] = 1 if c >= s else 0: keep-mask for diagonal 128-blocks."""
    sp = np.arange(128)[:, None]
    cp = np.arange(512)[None, :]
    return np.ascontiguousarray((cp >= sp).astype(np.float16))


def _emit(nc, tc, d, ctx):
    const = ctx.enter_context(tc.tile_pool(name="const", bufs=1))
    qkp = ctx.enter_context(tc.tile_pool(name="qk", bufs=1))
    vp = ctx.enter_context(tc.tile_pool(name="v", bufs=1))
    att = ctx.enter_context(tc.tile_pool(name="att", bufs=1))
    ptp = ctx.enter_context(tc.tile_pool(name="pt", bufs=4))
    tmp = ctx.enter_context(tc.tile_pool(name="tmp", bufs=3))
    small = ctx.enter_context(tc.tile_pool(name="small", bufs=2))
    stage = ctx.enter_context(tc.tile_pool(name="stage", bufs=4))
    # single PSUM pool, 8 banks total: "a" (qkv/v accum) 2, "s" (scores,
    # rope, warm-up) 4, "o0"/"o1" (attention-out accum, then proj) 2
    ps = ctx.enter_context(tc.tile_pool(name="ps", bufs=1, space="PSUM"))

    # ---- constants (DMA priority order: first-needed first, spread over
    #      the sync/scalar/gpsimd queues) ----
    warm_t = const.tile([128, 512], f16, tag="warm")
    nc.vector.memset(warm_t[:], 0.0)

    wqk_t = []
    for kc in range(KC):
        w1 = const.tile([128, 512], f16, tag=f"wqk{kc}", name=f"wqk{kc}")
        nc.sync.dma_start(w1[:], d["wqk"][128 * kc:128 * (kc + 1), :])
        wqk_t.append(w1)
    wv_t = []
    for kc in range(KC):
        w2 = const.tile([128, 256], f16, tag=f"wv{kc}", name=f"wv{kc}")
        nc.sync.dma_start(w2[:], d["wv"][128 * kc:128 * (kc + 1), :])
        wv_t.append(w2)

    rT_t = const.tile([128, 128], f16, tag="rT")
    csc_t = const.tile([128, T], f16, tag="csc")
    csn_t = const.tile([128, T], f16, tag="csn")
    tri_t = const.tile([128, 512], f16, tag="tri")
    wp_t = [const.tile([128, D], f16, tag=f"wp{kc}", name=f"wp{kc}")
            for kc in range(2)]
    nc.gpsimd.dma_start(rT_t[:], d["rT"][:])

    # x^T: first two t-chunks in [128, 512] pieces (they gate the first
    # matmuls), the second half in [128, 1024] pieces
    xT_t = [const.tile([128, T], f16, tag=f"xt{kc}", name=f"xt{kc}")
            for kc in range(KC)]
    for h2 in range(3):
        hsl2 = (slice(0, 512), slice(512, 1024), slice(1024, T))[h2]
        for kc in range(KC):
            dma_eng = nc.scalar if kc % 2 == 0 else nc.gpsimd
            dma_eng.dma_start(xT_t[kc][:, hsl2],
                              d["xT"][128 * kc:128 * (kc + 1), hsl2])
        nc.gpsimd.dma_start(csc_t[:, hsl2], d["csc"][:, hsl2])
        nc.gpsimd.dma_start(csn_t[:, hsl2], d["csn"][:, hsl2])
        if h2 == 0:
            nc.sync.dma_start(tri_t[:], d["tri"][:])
            for kc2 in range(2):
                nc.sync.dma_start(
                    wp_t[kc2][:], d["wp"][128 * kc2:128 * (kc2 + 1), :])

    # ---- persistent activations ----
    qk = [qkp.tile([128, T], f16, tag=f"qk{j}", name=f"qk{j}")
          for j in range(4)]
    # v layout per s-chunk: [V_h0 | ones | V_h1 | ones | ...] so the A@V
    # stationary [128, 128] = [V_h | ones]; the 64 ones columns make PSUM
    # rows 64-127 all equal to the softmax denominator (DVE-friendly).
    v_all = vp.tile([128, SC128 * 512], f16, tag="vall")
    nc.vector.memset(
        v_all.rearrange("p (n c) -> p n c", n=4 * SC128, c=128)[:, :, 64:128],
        1.0)
    attn = [att.tile([128, T], f16, tag=f"at{p}", name=f"at{p}")
            for p in range(NPAIR)]

    # warm-up matmuls: keep the PE array active (and the HAM clock
    # ungated) while the first x^T / weight DMAs stream in
    wps = ps.tile([128, 512], f32, tag="s", bufs=2, name="wps")
    for _ in range(WARM):
        nc.tensor.matmul(wps[:], warm_t[:, 0:128], warm_t[:],
                         start=True, stop=True)

    # ===== emission units (phase 1 and attention, interleaved below) ====
    def p1_qk(tcc, jp):
        """q/k projection block jp (2 of 4 qk rows) for t-chunk tcc + RoPE."""
        tsl = slice(512 * tcc, 512 * (tcc + 1))
        pss2 = [ps.tile([128, 512], f32, tag="a", bufs=2, name=f"ps{u}")
                for u in range(2)]
        for kc in range(KC):
            for u in range(2):
                jc = 2 * jp + u
                nc.tensor.matmul(pss2[u][:],
                                 wqk_t[kc][:, 128 * jc:128 * (jc + 1)],
                                 xT_t[kc][:, tsl],
                                 start=(kc == 0), stop=(kc == KC - 1))
        for u in range(2):
            jc = 2 * jp + u
            dst = qk[jc][:, tsl]
            nc.scalar.copy(dst, pss2[u][:])
            rps = ps.tile([128, 512], f32, tag="s", bufs=2, name="rps")
            nc.tensor.matmul(rps[:], rT_t[:], dst, start=True, stop=True)
            t1 = tmp.tile([128, 512], f32, tag="t1")
            nc.vector.tensor_mul(t1[:], rps[:], csn_t[:, tsl])
            t2 = tmp.tile([128, 512], f32, tag="t2")
            nc.gpsimd.tensor_mul(t2[:], dst, csc_t[:, tsl])
            nc.vector.tensor_add(dst, t1[:], t2[:])

    def p1_v(tcc):
        """V for the 4 s-chunks of t-chunk tcc, directly in [s, dh]."""
        for b4 in range(4):
            i = 4 * tcc + b4
            pv = ps.tile([128, 512], f32, tag="a", bufs=2, name="pv")
            for kc in range(KC):
                nc.tensor.matmul(pv[:, 0:256],
                                 xT_t[kc][:, 128 * i:128 * (i + 1)],
                                 wv_t[kc][:],
                                 start=(kc == 0), stop=(kc == KC - 1))
            nc.vector.tensor_copy(
                v_all[:, 512 * i:512 * (i + 1)].rearrange(
                    "p (a c) -> p a c", a=8, c=64)[:, 0::2, :],
                pv[:, 0:256].rearrange("p (a c) -> p a c", a=4, c=64))

    def pair(j, p):
        """Causal attention for head pair p over query strip j."""
        tsl = slice(512 * j, 512 * (j + 1))
        ni = 4 * (j + 1)
        qc = qk[2 * p]
        kch = qk[2 * p + 1]
        po = [ps.tile([128, 512], f32, tag=f"o{hh}", name=f"po{hh}")
              for hh in range(2)]

        def av(ptl, g):
            """A@V for one score group (emitted one group late)."""
            for hh in range(2):
                h = 2 * p + hh
                for half in range(2):
                    ii = 2 * g + half
                    diag = ii >= 4 * j
                    r = ii - 4 * j if diag else 0
                    c0 = 128 * r if diag else 0
                    nc.tensor.matmul(
                        po[hh][:, c0:512],
                        v_all[:, 512 * ii + 128 * h:512 * ii + 128 * (h + 1)],
                        ptl[hh][:, 512 * half + c0:512 * half + 512],
                        start=(ii == 0), stop=(ii == ni - 1))

        pending = []
        for g in range(ni // 2):
            pss = [ps.tile([128, 1024], f32, tag="s", bufs=2,
                           name=f"pss{hh}") for hh in range(2)]
            for half in range(2):
                ii = 2 * g + half
                diag = ii >= 4 * j
                r = ii - 4 * j if diag else 0
                c0 = 512 * half + (128 * r if diag else 0)
                qs = slice(512 * j + 128 * r, 512 * (j + 1)) \
                    if diag else tsl
                for hh in range(2):
                    hsl = slice(64 * hh, 64 * (hh + 1))
                    nc.tensor.matmul(
                        pss[hh][:, c0:512 * half + 512],
                        kch[hsl, 128 * ii:128 * (ii + 1)],
                        qc[hsl, qs],
                        start=True, stop=True)
            ptl = [ptp.tile([128, 1024], f16, tag=f"ptl{hh}",
                            name=f"ptl{hh}") for hh in range(2)]
            for hh in range(2):
                nc.scalar.activation(ptl[hh][:], pss[hh][:], EXP,
                                     scale=0.125)
            # zero the upper triangle of diagonal blocks (DVE)
            for half in range(2):
                ii = 2 * g + half
                if ii >= 4 * j:
                    r = ii - 4 * j
                    w = 512 - 128 * r
                    for hh in range(2):
                        reg = ptl[hh][:, 512 * half + 128 * r:
                                      512 * (half + 1)]
                        nc.vector.tensor_mul(reg, reg, tri_t[:, 0:w])
            pending.append((ptl, g))
            if len(pending) > 2:
                av(*pending.pop(0))
        for pe_ in pending:
            av(*pe_)

        for hh in range(2):
            sr = small.tile([64, 512], f32, tag="sr")
            nc.vector.tensor_copy(sr[:], po[hh][64:128, :])
            rc = small.tile([64, 512], f32, tag="rc")
            nc.vector.reciprocal_approx_fast(rc[:], sr[:])
            nc.vector.tensor_mul(
                attn[p][64 * hh:64 * (hh + 1), tsl],
                po[hh][0:64, :], rc[:])

    def proj(j):
        """Output projection (this core's partial) for strip j."""
        tsl = slice(512 * j, 512 * (j + 1))
        for oc in range(D // 128):
            pp = ps.tile([128, 512], f32, tag=f"o{oc % 2}", name="pp")
            for kc2 in range(2):
                nc.tensor.matmul(
                    pp[:], wp_t[kc2][:, 128 * oc:128 * (oc + 1)],
                    attn[kc2][:, tsl],
                    start=(kc2 == 0), stop=(kc2 == 1))
            ob = stage.tile([128, 512], f16, tag="ob")
            nc.vector.tensor_copy(ob[:], pp[:])
            out_eng = (nc.sync, nc.gpsimd, nc.scalar)[oc % 3]
            out_eng.dma_start(d["o"][128 * oc:128 * (oc + 1), tsl], ob[:])

    # ===== interleaved schedule: attention strip j runs between the =====
    # ===== projection blocks of t-chunk j+1 (PE work fills the gaps =====
    # ===== left by the Scalar engine's exp evictions, and vice versa) ===
    p1_qk(0, 0)
    p1_qk(0, 1)
    p1_v(0)
    for j in range(TC512):
        pair(j, 0)
        if j + 1 < TC512:
            p1_qk(j + 1, 0)
        pair(j, 1)
        if j + 1 < TC512:
            p1_qk(j + 1, 1)
        proj(j)
        if j + 1 < TC512:
            p1_v(j + 1)


def _build_module():
    nc = bacc.Bacc("TRN2", target_bir_lowering=False, debug=False,
                   enable_asserts=False)
    d = {
        "xT": nc.dram_tensor("xT", [D, T], f16, kind="ExternalInput").ap(),
        "wqk": nc.dram_tensor("wqk", [D, 512], f16, kind="ExternalInput").ap(),
        "wv": nc.dram_tensor("wv", [D, 256], f16, kind="ExternalInput").ap(),
        "wp": nc.dram_tensor("wp", [256, D], f16, kind="ExternalInput").ap(),
        "csc": nc.dram_tensor("csc", [128, T], f16, kind="ExternalInput").ap(),
        "csn": nc.dram_tensor("csn", [128, T], f16, kind="ExternalInput").ap(),
        "rT": nc.dram_tensor("rT", [128, 128], f16, kind="ExternalInput").ap(),
        "tri": nc.dram_tensor("tri", [128, 512], f16,
                              kind="ExternalInput").ap(),
        "o": nc.dram_tensor("o", [D, T], f16, kind="ExternalOutput").ap(),
    }
    with tile.TileContext(nc) as tc:
        with ExitStack() as ctx, \
             nc.allow_low_precision("fp16 PE operands are rounded by design"):
            _emit(nc, tc, d, ctx)
    nc.compile()
    return nc


def _get_module():
    if "nc" not in _CACHE:
        _CACHE["nc"] = _build_module()
    return _CACHE["nc"]


def _canonical(attn_mask, key_padding_mask):
    if attn_mask.shape != (1, 1, T, T) or key_padding_mask.shape != (B, T):
        return False
    if not key_padding_mask.all():
        return False
    m = np.asarray(attn_mask[0, 0], dtype=np.float32)
    causal = np.triu(np.full((T, T), -1e9, dtype=np.float32), k=1)
    return np.array_equal(m, causal)


def _reference_fallback(x, attn_mask, key_padding_mask, Wqkv, Wproj):
    x = np.asarray(x, np.float32)
    qkv = x @ np.asarray(Wqkv, np.float32).T
    q, k, v = qkv[..., :D], qkv[..., D:2 * D], qkv[..., 2 * D:]

    def split(t):
        return t.reshape(B, -1, H, DH).transpose(0, 2, 1, 3)

    def rope(xx):
        inv = 1.0 / (10000.0 ** (np.arange(0, DH, 2, dtype=np.float32) / DH))
        fr = np.arange(T, dtype=np.float32)[:, None] * inv[None, :]
        emb = np.concatenate([fr, fr], axis=-1)
        cos, sin = np.cos(emb)[None, None], np.sin(emb)[None, None]
        x1, x2 = xx[..., ::2], xx[..., 1::2]
        rh = np.stack((-x2, x1), axis=-1).reshape(xx.shape)
        return xx * cos + rh * sin

    q, k, v = split(q), split(k), split(v)
    q, k = rope(q), rope(k)
    s = np.einsum("bhtd,bhsd->bhts", q, k) / np.sqrt(np.float32(DH))
    s = s + np.asarray(attn_mask, np.float32)
    s = np.where(np.asarray(key_padding_mask)[:, None, None, :], s, -1e9)
    s = s - s.max(axis=-1, keepdims=True)
    e = np.exp(s)
    a = e / e.sum(axis=-1, keepdims=True)
    out = np.einsum("bhts,bhsd->bhtd", a, v)
    out = out.transpose(0, 2, 1, 3).reshape(B, T, D)
    return out @ np.asarray(Wproj, np.float32).T


def _make_in_maps(x, Wqkv, Wproj):
    csc, csn = _rope_tables()
    rT = _rot_matrix().astype(np.float16)
    tri = _tri_mask()

    Wq = np.asarray(Wqkv[:D], np.float32).reshape(H, DH, D)
    Wk = np.asarray(Wqkv[D:2 * D], np.float32).reshape(H, DH, D)
    Wv = np.asarray(Wqkv[2 * D:], np.float32).reshape(H, DH, D)
    WpT = np.ascontiguousarray(np.asarray(Wproj, np.float32).T)  # [din, dout]

    xT = [np.ascontiguousarray(np.asarray(x[b], np.float32).T.astype(np.float16))
          for b in range(B)]

    in_maps = []
    for c in range(NCORES):
        b, g = divmod(c, GROUPS)
        hs = [HPC * g + hl for hl in range(HPC)]  # global head ids
        cols = []
        for pp in range(NPAIR):
            h0, h1 = hs[2 * pp], hs[2 * pp + 1]
            cols.append(np.concatenate([Wq[h0], Wq[h1]], axis=0))  # [128, D]
            cols.append(np.concatenate([Wk[h0], Wk[h1]], axis=0))
        wqk = np.ascontiguousarray(
            np.concatenate(cols, axis=0).T.astype(np.float16))     # [D, 512]
        wv = np.ascontiguousarray(
            np.concatenate([Wv[h] for h in hs], axis=0).T.astype(np.float16))
        wp = np.ascontiguousarray(
            WpT[256 * g:256 * (g + 1), :].astype(np.float16))  # [256, D]
        in_maps.append({
            "xT": xT[b], "wqk": wqk, "wv": wv, "wp": wp,
            "csc": csc, "csn": csn, "rT": rT, "tri": tri,
        })
    return in_maps


def _in_maps_for_trace(inputs):
    return _make_in_maps(np.asarray(inputs["x"]), np.asarray(inputs["Wqkv"]),
                         np.asarray(inputs["Wproj"]))


def kernel(x, attn_mask, key_padding_mask, Wqkv, Wproj):
    x = np.asarray(x)
    attn_mask = np.asarray(attn_mask)
    key_padding_mask = np.asarray(key_padding_mask)
    Wqkv = np.asarray(Wqkv)
    Wproj = np.asarray(Wproj)

    if not _canonical(attn_mask, key_padding_mask):
        return _reference_fallback(x, attn_mask, key_padding_mask, Wqkv, Wproj)

    nc = _get_module()
    in_maps = _make_in_maps(x, Wqkv, Wproj)
    res = bass_utils.run_bass_kernel_spmd(nc, in_maps,
                                          core_ids=list(range(NCORES)))
    out = np.empty((B, T, D), dtype=np.float32)
    for b in range(B):
        acc = res.results[4 * b]["o"].astype(np.float32)
        for g in range(1, GROUPS):
            acc += res.results[4 * b + g]["o"].astype(np.float32)
        out[b] = acc.T
    return out
